# revision 16
# baseline (speedup 1.0000x reference)
"""Trainium2 Bass kernel for causal self-attention (B=4, T=2048, C=2048, H=16).

Sharding: 8 cores = DP4 (batch) x TP2 (8 heads each). Each core:
  P1  qk' = (x @ Wqk)^T computed directly in [j, t] layout (j = head-dim rows),
      kept resident in SBUF (no DRAM round trip).
  P2  v   = x @ Wv in natural [t, j] layout (stationary = xT tiles), with the
      Wv weights streamed per 256-col range (double-buffered).
  P3/P4 interleaved per query-chunk ic (512 queries):
      P3  per head, key-blocks processed in PAIRS: two s matmuls into one
          2-bank PSUM tile -> mask -> ONE exp over [128,1024] (amortizes the
          ACT fixed cost) -> po += v att (per block); att sums accumulate on
          the DVE in bf16; one ones-matmul per pass turns the acc into the
          softmax denominator pd; y = po * recip_fast(pd).
      P4  partial[tb, :] = y'^T @ Wp + bias (even core), stored bf16
      RS  per half-quarter ReduceScatter(add) over the core pair, output in
          Shared DRAM, copied to the external output. The collectives
          pipeline behind the next compute.

All matmuls bf16 with fp32 PSUM accumulation; softmax in fp32 on ACT/DVE.
Host side: shard/cast/permute inputs (per-block contiguous weight layouts so
each weight block loads with ONE fully-contiguous DMA), assemble output
(bf16 -> f32).
"""
import os
import math
import numpy as np
import ml_dtypes

import concourse.bass as bass
import concourse.bacc as bacc
import concourse.mybir as mybir
import concourse.tile as tile

F32 = mybir.dt.float32
BF16 = mybir.dt.bfloat16
AF = mybir.ActivationFunctionType

D = 128          # head dim (fixed: partition size)
N_CORES = 8
PAIRS = [[0, 1], [2, 3], [4, 5], [6, 7]]


class Cfg:
    def __init__(self, T=2048, H_TOT=16, HPC=8, B=4):
        self.T = T                    # sequence length
        self.H_TOT = H_TOT            # total heads
        self.HPC = HPC                # heads per core
        self.B = B
        self.C = H_TOT * D            # model dim
        self.CP = HPC * D             # per-core head cols
        self.TCH = 512                # ti chunk width
        assert T % self.TCH == 0 and T % D == 0


def build_kernel(cfg: Cfg):
    T, C, CP, HPC, TCH = cfg.T, cfg.C, cfg.CP, cfg.HPC, cfg.TCH
    NC_CH = C // D                # c-chunks (contraction)
    NJB = 2 * HPC                 # qk' j-blocks (q/k interleaved per head)
    NTB = T // D                  # t-blocks
    NIC = T // TCH                # ti chunks (query quarters)
    NTR = T // 512                # t-ranges for P1 moving dim
    NPR = C // 512                # proj n-ranges
    VRW = 256                     # v col range width
    NVR = CP // VRW               # v col ranges
    NXG = 8                       # xT DMA groups
    XGW = NC_CH // NXG            # c-chunks per xT DMA group
    TBQ = TCH // D                # t-blocks per quarter
    RQ = TCH // 2                 # rows each core owns per quarter after RS
    HQ = TCH // 2                 # rows per half-quarter RS input
    scale = 1.0 / math.sqrt(D)

    nc = bacc.Bacc()
    # per-block contiguous layouts: each SBUF block is one contiguous
    # [128, n] DMA (4KB+ per partition line)
    xT = nc.declare_dram_parameter("xT", [D, NC_CH * T], BF16, isOutput=False)
    wqk = nc.declare_dram_parameter("wqk", [NJB, D, NC_CH * D], BF16,
                                    isOutput=False)
    wv = nc.declare_dram_parameter("wv", [NVR, D, NC_CH * VRW], BF16,
                                   isOutput=False)
    wp = nc.declare_dram_parameter("wp", [D, HPC * C], BF16, isOutput=False)
    bqk = nc.declare_dram_parameter("bqk", [D, NJB], F32, isOutput=False)
    bv = nc.declare_dram_parameter("bv", [D, CP], BF16, isOutput=False)
    bp = nc.declare_dram_parameter("bp", [D, C], BF16, isOutput=False)
    masks = nc.declare_dram_parameter("masks", [D, 4 * TCH], BF16, isOutput=False)
    out_ext = nc.declare_dram_parameter("out", [NIC * RQ, C], BF16, isOutput=True)

    partial_q = [nc.dram_tensor(f"partial_{q}", [TCH, C], BF16)
                 for q in range(NIC)]
    rs_q = [[nc.dram_tensor(f"rs_{q}_{hf}", [HQ // 2, C], BF16)
             for hf in range(2)] for q in range(NIC)]

    with tile.TileContext(nc) as tc:
        with (
            tc.tile_pool(name="const", bufs=1) as constp,
            tc.tile_pool(name="vres", bufs=1) as vres,
            tc.tile_pool(name="yres", bufs=16) as yres,
            tc.tile_pool(name="qkres", bufs=1) as qkres,
        ):
            # ---------------- P1: qk' ----------------
            # first j-block's weights before the big x load so the PE can
            # start as soon as xT group 0 lands (pool open order is LIFO:
            # xtp outlives wqkp)
            xtp_ctx = tc.tile_pool(name="xtp", bufs=1)
            xtp = xtp_ctx.__enter__()
            wqkp_ctx = tc.tile_pool(name="wqkp", bufs=2)
            wqkp = wqkp_ctx.__enter__()

            def load_wt(jb):
                # one fully-contiguous DMA: all 16 c-chunks of j-block jb
                wt = wqkp.tile([D, NC_CH * D], BF16, name="wt", tag="wt")
                nc.sync.dma_start(wt[:], wqk[jb, :, :])
                return wt

            wt0 = load_wt(0)

            bqk_t = constp.tile([D, NJB], F32, name="bqk_t")
            nc.scalar.dma_start(bqk_t[:], bqk[:, :])

            # resident xT (bf16) as one tile, loaded in NXG contiguous DMAs
            # on the scalar queue -- hardware DGE, parallel with the weight
            # DMAs on the sync queue (the gpsimd queue uses the slow SW DGE)
            xt_all = xtp.tile([D, NC_CH * T], BF16, name="xt_all")
            for g in range(NXG):
                lo, hi = g * XGW * T, (g + 1) * XGW * T
                nc.scalar.dma_start(xt_all[:, lo:hi], xT[:, lo:hi])

            # biases pre-broadcast across partitions (host-side) so PSUM
            # evacuation can fuse the bias add on the DVE (replaces the
            # ones-row bias matmuls)
            bvb = constp.tile([D, CP], BF16, name="bvb")
            nc.scalar.dma_start(bvb[:], bv[:, :])
            bpb = constp.tile([D, C], BF16, name="bpb")
            nc.scalar.dma_start(bpb[:], bp[:, :])
            # keep/kill masks (1.0/0.0), applied multiplicatively post-exp:
            # partition dim must be D -> load as [D, 4*TCH]
            mask_sb = constp.tile([D, 4 * TCH], BF16, name="mask_sb")
            nc.scalar.dma_start(mask_sb[:], masks[:, :])
            ones_sq = constp.tile([D, D], BF16, name="ones_sq")
            nc.vector.memset(ones_sq[:], 1.0)

            # persistent qk' tiles [D, T] per j-block
            qk_sb = [qkres.tile([D, T], BF16, name=f"qk{jb}")
                     for jb in range(NJB)]
            with tc.tile_pool(name="pq", bufs=8, space="PSUM") as pqp:
                for jb in range(NJB):
                    wt = wt0 if jb == 0 else load_wt(jb)
                    ps = [pqp.tile([D, 512], F32, name="pq", tag="pq")
                          for _ in range(NTR)]
                    for c in range(NC_CH):
                        for tr in range(NTR):
                            nc.tensor.matmul(
                                ps[tr][:], wt[:, c * D:(c + 1) * D],
                                xt_all[:, c * T + tr * 512:c * T + (tr + 1) * 512],
                                start=(c == 0), stop=(c == NC_CH - 1))
                    for tr in range(NTR):
                        nc.vector.tensor_scalar_add(
                            qk_sb[jb][:, tr * 512:(tr + 1) * 512], ps[tr][:],
                            bqk_t[:, jb:jb + 1])
            wqkp_ctx.__exit__(None, None, None)

            # ---------------- P2: v ----------------
            # v_sb[tb] is [t, j]; Wv streamed per VRW-col range, double-buffered
            v_sb = [vres.tile([D, CP], BF16, name=f"v{tb}")
                    for tb in range(NTB)]
            with (
                tc.tile_pool(name="wvp", bufs=2) as wvp,
                tc.tile_pool(name="pv", bufs=4, space="PSUM") as pvp,
            ):
                for vr in range(NVR):
                    wv_t = wvp.tile([D, NC_CH * VRW], BF16, name="wv",
                                    tag="wv")
                    nc.scalar.dma_start(wv_t[:], wv[vr, :, :])
                    for tb in range(NTB):
                        pv = pvp.tile([D, VRW], F32, name="pv", tag="pv")
                        for c in range(NC_CH):
                            nc.tensor.matmul(
                                pv[:], xt_all[:, c * T + tb * D:c * T + (tb + 1) * D],
                                wv_t[:, c * VRW:(c + 1) * VRW],
                                start=(c == 0), stop=(c == NC_CH - 1))
                        nc.vector.tensor_add(
                            v_sb[tb][:, vr * VRW:(vr + 1) * VRW], pv[:],
                            bvb[:, vr * VRW:(vr + 1) * VRW])
            xtp_ctx.__exit__(None, None, None)

            # ---------------- P3 + P4 + RS interleaved per quarter --------
            with (
                tc.tile_pool(name="wpp", bufs=1) as wpp,
                tc.tile_pool(name="attp", bufs=4) as attp,
                tc.tile_pool(name="accp", bufs=2) as accp,
                tc.tile_pool(name="ps_s", bufs=2, space="PSUM") as ps_s,
                tc.tile_pool(name="ps_o", bufs=2, space="PSUM") as ps_o,
                tc.tile_pool(name="pp", bufs=2, space="PSUM") as ppp,
                tc.tile_pool(name="normp", bufs=2) as normp,
                tc.tile_pool(name="post", bufs=3) as post,
            ):
                # prefetch proj weights now that xT is freed (one DMA)
                wp_all = wpp.tile([D, HPC * C], BF16, name="wp_all")
                nc.scalar.dma_start(wp_all[:], wp[:, :])

                for ic in range(NIC):
                    ti0 = ic * TCH
                    ntk = (ti0 + TCH) // D
                    npair = ntk // 2
                    # per-quarter y tiles (consumed by P4 right below)
                    y_sb = [yres.tile([D, TCH], BF16, name="y", tag="y")
                            for _ in range(HPC)]
                    for h in range(HPC):
                        qp = qk_sb[2 * h]
                        kp = qk_sb[2 * h + 1]
                        po = ps_o.tile([D, TCH], F32, name="po", tag="po")
                        acc = accp.tile([D, 2 * TCH], BF16, name="acc",
                                        tag="acc")
                        for pr in range(npair):
                            tk0 = 2 * pr
                            # two s blocks into one 2-bank PSUM tile
                            s2 = ps_s.tile([D, 2 * TCH], F32, name="s2",
                                           tag="s2")
                            for u in range(2):
                                tk = tk0 + u
                                nc.tensor.matmul(
                                    s2[:, u * TCH:(u + 1) * TCH],
                                    kp[:, tk * D:(tk + 1) * D],
                                    qp[:, ti0:ti0 + TCH],
                                    start=True, stop=True)
                            att2 = attp.tile([D, 2 * TCH], BF16, name="att",
                                             tag="att")
                            nc.scalar.activation(att2[:], s2[:], AF.Exp,
                                                 bias=0.0, scale=scale)
                            # causal masking: multiplicative 0/1 mask after
                            # exp (bf16 2x DVE rate, and off the s->exp
                            # critical path)
                            kd0 = tk0 - (ntk - TCH // D)
                            if kd0 >= -1:
                                if kd0 >= 0:
                                    nc.vector.tensor_mul(
                                        att2[:], att2[:],
                                        mask_sb[:, kd0 * TCH:(kd0 + 2) * TCH])
                                else:
                                    nc.vector.tensor_mul(
                                        att2[:, TCH:], att2[:, TCH:],
                                        mask_sb[:, 0:TCH])
                            # att column-sum accumulation on DVE (bf16 2x)
                            if pr == 0:
                                nc.vector.tensor_copy(acc[:], att2[:])
                            else:
                                nc.vector.tensor_add(acc[:], acc[:], att2[:])
                            for u in range(2):
                                tk = tk0 + u
                                nc.tensor.matmul(
                                    po[:], v_sb[tk][:, h * D:(h + 1) * D],
                                    att2[:, u * TCH:(u + 1) * TCH],
                                    start=(tk == 0), stop=(tk == ntk - 1))
                        # denominator: one ones-matmul over the accumulated
                        # att sums (contract the 128 acc partitions)
                        pd = ppp.tile([D, TCH], F32, name="pd", tag="pp")
                        nc.tensor.matmul(pd[:], ones_sq[:], acc[:, 0:TCH],
                                         start=True, stop=False)
                        nc.tensor.matmul(pd[:], ones_sq[:], acc[:, TCH:],
                                         start=False, stop=True)
                        recb = normp.tile([D, TCH], F32, name="recb",
                                          tag="recb")
                        nc.vector.reciprocal_approx_fast(recb[:], pd[:])
                        nc.vector.tensor_mul(y_sb[h][:], po[:], recb[:])

                    # ---- P4 for this quarter ----
                    for tq in range(TBQ):
                        pst = post.tile([D, C], BF16, name="pst", tag="pst")
                        for nr in range(NPR):
                            pp = ppp.tile([D, 512], F32, name="pp", tag="pp")
                            for c in range(HPC):
                                nc.tensor.matmul(
                                    pp[:], y_sb[c][:, tq * D:(tq + 1) * D],
                                    wp_all[:, c * C + nr * 512:c * C + (nr + 1) * 512],
                                    start=(c == 0), stop=(c == HPC - 1))
                            nc.vector.tensor_add(
                                pst[:, nr * 512:(nr + 1) * 512], pp[:],
                                bpb[:, nr * 512:(nr + 1) * 512])
                        nc.sync.dma_start(
                            partial_q[ic][tq * D:(tq + 1) * D, :], pst[:])
                        # ---- half-quarter ReduceScatter: issue as soon as
                        # the 2 contributing row-blocks are stored ----
                        if tq % 2 == 1:
                            hf = tq // 2
                            nc.gpsimd.collective_compute(
                                "ReduceScatter",
                                mybir.AluOpType.add,
                                ins=[partial_q[ic][hf * HQ:(hf + 1) * HQ, :]],
                                outs=[rs_q[ic][hf][:, :]],
                                replica_groups=PAIRS,
                            )
                            nc.sync.dma_start(
                                out_ext[ic * RQ + hf * (HQ // 2):
                                        ic * RQ + (hf + 1) * (HQ // 2), :],
                                rs_q[ic][hf][:, :])
    nc.finalize()
    return nc


def _prep_inputs(cfg: Cfg, x, w_attn, b_attn, w_proj, b_proj):
    """Host-side shard/cast/permute. Returns in_maps (list of dicts per core).

    Weight/x layouts are per-block contiguous so that each on-chip block
    loads with a single fully-contiguous DMA.
    """
    T, C, CP, HPC = cfg.T, cfg.C, cfg.CP, cfg.HPC
    NC_CH = C // D
    NJB = 2 * HPC
    VRW = 256
    NVR = CP // VRW
    bf = ml_dtypes.bfloat16
    wq = w_attn[:, 0:C]
    wk = w_attn[:, C:2 * C]
    wvf = w_attn[:, 2 * C:3 * C]
    bq, bk, bvf = b_attn[0:C], b_attn[C:2 * C], b_attn[2 * C:3 * C]

    masks = np.zeros((D, 4 * cfg.TCH), dtype=bf)
    f = np.arange(cfg.TCH)[None, :]
    p = np.arange(D)[:, None]
    for k in range(4):
        keep = (f - p >= 128 * k)
        masks[:, k * cfg.TCH:(k + 1) * cfg.TCH] = np.where(
            keep, 1.0, 0.0).astype(bf)

    in_maps = []
    for core in range(N_CORES):
        b = core // 2
        g = core % 2
        h0 = g * HPC * D            # first col of this head group
        sl = slice(h0, h0 + CP)
        # x[b].T in chunk-contiguous layout: [D, NC_CH*T],
        # xT_p[p, c*T + t] = x[b][t, c*D + p]
        xTc = np.ascontiguousarray(
            x[b].reshape(T, NC_CH, D).transpose(2, 1, 0)).astype(bf)
        xTc = xTc.reshape(D, NC_CH * T)
        wqk_cols = []
        for h in range(HPC):
            hs = slice(h0 + h * D, h0 + (h + 1) * D)
            wqk_cols.append(wq[:, hs])
            wqk_cols.append(wk[:, hs])
        # [C, NJB*D] -> [NJB, D, NC_CH*D]:
        # wqk_c[jb, p, c*D + j] = w[c*D + p, jb*D + j]
        wqk_cat = np.concatenate(wqk_cols, axis=1)
        wqk_c = np.ascontiguousarray(
            wqk_cat.reshape(NC_CH, D, NJB, D).transpose(2, 1, 0, 3)
        ).reshape(NJB, D, NC_CH * D).astype(bf)
        # wv: [C, CP] -> [NVR, D, NC_CH*VRW]
        wv_c = np.ascontiguousarray(
            wvf[:, sl].reshape(NC_CH, D, NVR, VRW).transpose(2, 1, 0, 3)
        ).reshape(NVR, D, NC_CH * VRW).astype(bf)
        # wp: [CP, C] -> [D, HPC*C], wp_p[p, c*C + n] = wp[c*D + p, n]
        wp_c = np.ascontiguousarray(
            w_proj[sl, :].reshape(HPC, D, C).transpose(1, 0, 2)).astype(bf)
        wp_c = wp_c.reshape(D, HPC * C)
        bqk_cols = []
        for h in range(HPC):
            hs = slice(h0 + h * D, h0 + (h + 1) * D)
            bqk_cols.append(bq[hs])
            bqk_cols.append(bk[hs])
        bqk_c = np.ascontiguousarray(np.stack(bqk_cols, axis=1)).astype(np.float32)
        in_maps.append({
            "xT": xTc,
            "wqk": wqk_c,
            "wv": wv_c,
            "wp": wp_c,
            "bqk": bqk_c,
            "bv": np.broadcast_to(
                bvf[sl].reshape(1, CP), (D, CP)).astype(bf),
            "bp": np.broadcast_to(
                (b_proj * (1.0 - g)).reshape(1, C), (D, C)).astype(bf),
            "masks": masks,
        })
    return in_maps


_CFG = Cfg()


def kernel(x, w_attn, b_attn, w_proj, b_proj, _trace=False, _cfg=None):
    from concourse.bass_utils import run_bass_kernel_spmd
    cfg = _cfg or _CFG
    x = np.asarray(x, dtype=np.float32)
    w_attn = np.asarray(w_attn, dtype=np.float32)
    b_attn = np.asarray(b_attn, dtype=np.float32)
    w_proj = np.asarray(w_proj, dtype=np.float32)
    b_proj = np.asarray(b_proj, dtype=np.float32)

    in_maps = _prep_inputs(cfg, x, w_attn, b_attn, w_proj, b_proj)
    nc = build_kernel(cfg)
    res = run_bass_kernel_spmd(nc, in_maps, list(range(N_CORES)), trace=_trace)
    # out rows per core: quarter q, half hf, rank g ->
    #   global rows q*512 + hf*256 + g*128 + [0:128)
    RQ = cfg.TCH // 2
    HQH = RQ // 2  # 128
    outs = []
    for b in range(cfg.B):
        even = res.results[2 * b]["out"].astype(np.float32)
        odd = res.results[2 * b + 1]["out"].astype(np.float32)
        ob = np.empty((cfg.T, cfg.C), dtype=np.float32)
        for q in range(cfg.T // cfg.TCH):
            for hf in range(2):
                r0 = q * RQ + hf * HQH
                g0 = q * cfg.TCH + hf * 2 * HQH
                ob[g0:g0 + HQH] = even[r0:r0 + HQH]
                ob[g0 + HQH:g0 + 2 * HQH] = odd[r0:r0 + HQH]
        outs.append(ob)
    full = np.stack(outs, axis=0)
    if _trace:
        kernel.last_exec_time_ns = res.exec_time_ns
        kernel.last_mean_exec_time_ns = res.mean_exec_time_ns
        kernel.last_scope_times = res.per_core_scope_times
        kernel.last_trace_path = (res.instructions_and_trace[1]
                                  if res.instructions_and_trace else None)
        kernel.last_insts = (res.instructions_and_trace[0]
                             if res.instructions_and_trace else None)
    return full


# revision 20
# speedup vs baseline: 1.2982x; 1.2982x over previous
"""Trainium2 Bass kernel for causal self-attention (B=4, T=2048, C=2048, H=16).

Sharding: 8 cores = DP4 (batch) x TP2 (8 heads each). Each core:
  P1  qk' = (x @ Wqk)^T computed directly in [j, t] layout (j = head-dim rows),
      kept resident in SBUF (no DRAM round trip).
  P2  v   = x @ Wv in natural [t, j] layout (stationary = xT tiles), with the
      Wv weights streamed per 256-col range (double-buffered).
  P3/P4 interleaved per query-chunk ic (512 queries):
      P3  per head, key-blocks processed in PAIRS: two s matmuls into one
          2-bank PSUM tile -> mask -> ONE exp over [128,1024] (amortizes the
          ACT fixed cost) -> po += v att (per block); att sums accumulate on
          the DVE in bf16; one ones-matmul per pass turns the acc into the
          softmax denominator pd; y = po * recip_fast(pd).
      P4  partial[tb, :] = y'^T @ Wp + bias (even core), stored bf16
      RS  per half-quarter ReduceScatter(add) over the core pair, output in
          Shared DRAM, copied to the external output. The collectives
          pipeline behind the next compute.

All matmuls bf16 with fp32 PSUM accumulation; softmax in fp32 on ACT/DVE.
Host side: shard/cast/permute inputs (per-block contiguous weight layouts so
each weight block loads with ONE fully-contiguous DMA), assemble output
(bf16 -> f32).
"""
import os
import math
import numpy as np
import ml_dtypes

import concourse.bass as bass
import concourse.bacc as bacc
import concourse.mybir as mybir
import concourse.tile as tile

F32 = mybir.dt.float32
BF16 = mybir.dt.bfloat16
AF = mybir.ActivationFunctionType

D = 128          # head dim (fixed: partition size)
N_CORES = 8
PAIRS = [[0, 1], [2, 3], [4, 5], [6, 7]]


class Cfg:
    def __init__(self, T=2048, H_TOT=16, HPC=8, B=4):
        self.T = T                    # sequence length
        self.H_TOT = H_TOT            # total heads
        self.HPC = HPC                # heads per core
        self.B = B
        self.C = H_TOT * D            # model dim
        self.CP = HPC * D             # per-core head cols
        self.TCH = 512                # ti chunk width
        assert T % self.TCH == 0 and T % D == 0


def build_kernel(cfg: Cfg):
    T, C, CP, HPC, TCH = cfg.T, cfg.C, cfg.CP, cfg.HPC, cfg.TCH
    NC_CH = C // D                # c-chunks (contraction)
    NJB = 2 * HPC                 # qk' j-blocks (q/k interleaved per head)
    NTB = T // D                  # t-blocks
    NIC = T // TCH                # ti chunks (query quarters)
    NTR = T // 512                # t-ranges for P1 moving dim
    NPR = C // 512                # proj n-ranges
    VRW = 256                     # v col range width
    NVR = CP // VRW               # v col ranges
    NXG = 8                       # xT DMA groups
    XGW = NC_CH // NXG            # c-chunks per xT DMA group
    TBQ = TCH // D                # t-blocks per quarter
    RQ = TCH // 2                 # rows each core owns per quarter after RS
    HQ = TCH // 2                 # rows per half-quarter RS input
    scale = 1.0 / math.sqrt(D)

    nc = bacc.Bacc()
    # per-block contiguous layouts: each SBUF block is one contiguous
    # [128, n] DMA (4KB+ per partition line)
    xT = nc.declare_dram_parameter("xT", [D, NC_CH * T], BF16, isOutput=False)
    wqk = nc.declare_dram_parameter("wqk", [NJB, D, NC_CH * D], BF16,
                                    isOutput=False)
    wv = nc.declare_dram_parameter("wv", [NVR, D, NC_CH * VRW], BF16,
                                   isOutput=False)
    wp = nc.declare_dram_parameter("wp", [D, HPC * C], BF16, isOutput=False)
    bqk = nc.declare_dram_parameter("bqk", [D, NJB], F32, isOutput=False)
    bv = nc.declare_dram_parameter("bv", [D, CP], BF16, isOutput=False)
    bp = nc.declare_dram_parameter("bp", [D, C], BF16, isOutput=False)
    masks = nc.declare_dram_parameter("masks", [D, 4 * TCH], BF16, isOutput=False)
    out_ext = nc.declare_dram_parameter("out", [NIC * RQ, C], BF16, isOutput=True)

    partial_q = [nc.dram_tensor(f"partial_{q}", [TCH, C], BF16)
                 for q in range(NIC)]
    rs_q = [[nc.dram_tensor(f"rs_{q}_{hf}", [HQ // 2, C], BF16)
             for hf in range(2)] for q in range(NIC)]

    with tile.TileContext(nc) as tc:
        with (
            tc.tile_pool(name="const", bufs=1) as constp,
            tc.tile_pool(name="vres", bufs=1) as vres,
            tc.tile_pool(name="yres", bufs=16) as yres,
            tc.tile_pool(name="qkres", bufs=1) as qkres,
        ):
            # ---------------- P1: qk' ----------------
            # first j-block's weights before the big x load so the PE can
            # start as soon as xT group 0 lands (pool open order is LIFO:
            # xtp outlives wqkp)
            xtp_ctx = tc.tile_pool(name="xtp", bufs=1)
            xtp = xtp_ctx.__enter__()
            wqkp_ctx = tc.tile_pool(name="wqkp", bufs=2)
            wqkp = wqkp_ctx.__enter__()

            def load_wt(jb):
                # one fully-contiguous DMA: all 16 c-chunks of j-block jb
                wt = wqkp.tile([D, NC_CH * D], BF16, name="wt", tag="wt")
                nc.sync.dma_start(wt[:], wqk[jb, :, :])
                return wt

            wt0 = load_wt(0)

            bqk_t = constp.tile([D, NJB], F32, name="bqk_t")
            nc.scalar.dma_start(bqk_t[:], bqk[:, :])

            # resident xT (bf16) as one tile, loaded in NXG contiguous DMAs
            # on the scalar queue -- hardware DGE, parallel with the weight
            # DMAs on the sync queue (the gpsimd queue uses the slow SW DGE)
            xt_all = xtp.tile([D, NC_CH * T], BF16, name="xt_all")
            for g in range(NXG):
                lo, hi = g * XGW * T, (g + 1) * XGW * T
                nc.scalar.dma_start(xt_all[:, lo:hi], xT[:, lo:hi])

            # biases pre-broadcast across partitions (host-side) so PSUM
            # evacuation can fuse the bias add on the DVE (replaces the
            # ones-row bias matmuls)
            bvb = constp.tile([D, CP], BF16, name="bvb")
            nc.scalar.dma_start(bvb[:], bv[:, :])
            bpb = constp.tile([D, C], BF16, name="bpb")
            nc.scalar.dma_start(bpb[:], bp[:, :])
            # keep/kill masks (1.0/0.0), applied multiplicatively post-exp:
            # partition dim must be D -> load as [D, 4*TCH]
            mask_sb = constp.tile([D, 4 * TCH], BF16, name="mask_sb")
            nc.scalar.dma_start(mask_sb[:], masks[:, :])
            ones_sq = constp.tile([D, D], BF16, name="ones_sq")
            nc.vector.memset(ones_sq[:], 1.0)

            # persistent qk' tiles [D, T] per j-block
            qk_sb = [qkres.tile([D, T], BF16, name=f"qk{jb}")
                     for jb in range(NJB)]
            with tc.tile_pool(name="pq", bufs=8, space="PSUM") as pqp:
                for jb in range(NJB):
                    wt = wt0 if jb == 0 else load_wt(jb)
                    ps = [pqp.tile([D, 512], F32, name="pq", tag="pq")
                          for _ in range(NTR)]
                    for c in range(NC_CH):
                        for tr in range(NTR):
                            nc.tensor.matmul(
                                ps[tr][:], wt[:, c * D:(c + 1) * D],
                                xt_all[:, c * T + tr * 512:c * T + (tr + 1) * 512],
                                start=(c == 0), stop=(c == NC_CH - 1))
                    for tr in range(NTR):
                        nc.vector.tensor_scalar_add(
                            qk_sb[jb][:, tr * 512:(tr + 1) * 512], ps[tr][:],
                            bqk_t[:, jb:jb + 1])
            wqkp_ctx.__exit__(None, None, None)

            # ---------------- P2: v ----------------
            # v_sb[tb] is [t, j]; Wv streamed per VRW-col range, double-buffered
            v_sb = [vres.tile([D, CP], BF16, name=f"v{tb}")
                    for tb in range(NTB)]
            with (
                tc.tile_pool(name="wvp", bufs=2) as wvp,
                tc.tile_pool(name="pv", bufs=4, space="PSUM") as pvp,
            ):
                for vr in range(NVR):
                    wv_t = wvp.tile([D, NC_CH * VRW], BF16, name="wv",
                                    tag="wv")
                    nc.scalar.dma_start(wv_t[:], wv[vr, :, :])
                    for tb in range(NTB):
                        pv = pvp.tile([D, VRW], F32, name="pv", tag="pv")
                        for c in range(NC_CH):
                            nc.tensor.matmul(
                                pv[:], xt_all[:, c * T + tb * D:c * T + (tb + 1) * D],
                                wv_t[:, c * VRW:(c + 1) * VRW],
                                start=(c == 0), stop=(c == NC_CH - 1))
                        nc.vector.tensor_add(
                            v_sb[tb][:, vr * VRW:(vr + 1) * VRW], pv[:],
                            bvb[:, vr * VRW:(vr + 1) * VRW])
            xtp_ctx.__exit__(None, None, None)

            # ---------------- P3 + P4 + RS interleaved per quarter --------
            with (
                tc.tile_pool(name="wpp", bufs=1) as wpp,
                tc.tile_pool(name="attp", bufs=4) as attp,
                tc.tile_pool(name="accp", bufs=2) as accp,
                tc.tile_pool(name="ps_s", bufs=2, space="PSUM") as ps_s,
                tc.tile_pool(name="ps_o", bufs=2, space="PSUM") as ps_o,
                tc.tile_pool(name="pp", bufs=2, space="PSUM") as ppp,
                tc.tile_pool(name="normp", bufs=2) as normp,
                tc.tile_pool(name="post", bufs=3) as post,
            ):
                # prefetch proj weights now that xT is freed (one DMA)
                wp_all = wpp.tile([D, HPC * C], BF16, name="wp_all")
                nc.scalar.dma_start(wp_all[:], wp[:, :])

                # out-copies (rs DRAM -> out_ext) wait on their RS; issuing
                # them a quarter late keeps that wait from blocking the next
                # quarter's pst stores on the same queue
                pending_copies = []
                for ic in range(NIC):
                    for (dst, src) in pending_copies:
                        nc.sync.dma_start(dst, src)
                    pending_copies = []
                    ti0 = ic * TCH
                    ntk = (ti0 + TCH) // D
                    npair = ntk // 2
                    # per-quarter y tiles (consumed by P4 right below)
                    y_sb = [yres.tile([D, TCH], BF16, name="y", tag="y")
                            for _ in range(HPC)]
                    for h in range(HPC):
                        qp = qk_sb[2 * h]
                        kp = qk_sb[2 * h + 1]
                        po = ps_o.tile([D, TCH], F32, name="po", tag="po")
                        acc = accp.tile([D, 2 * TCH], BF16, name="acc",
                                        tag="acc")
                        for pr in range(npair):
                            tk0 = 2 * pr
                            # two s blocks into one 2-bank PSUM tile
                            s2 = ps_s.tile([D, 2 * TCH], F32, name="s2",
                                           tag="s2")
                            for u in range(2):
                                tk = tk0 + u
                                nc.tensor.matmul(
                                    s2[:, u * TCH:(u + 1) * TCH],
                                    kp[:, tk * D:(tk + 1) * D],
                                    qp[:, ti0:ti0 + TCH],
                                    start=True, stop=True)
                            att2 = attp.tile([D, 2 * TCH], BF16, name="att",
                                             tag="att")
                            nc.scalar.activation(att2[:], s2[:], AF.Exp,
                                                 bias=0.0, scale=scale)
                            # causal masking: multiplicative 0/1 mask after
                            # exp (bf16 2x DVE rate, and off the s->exp
                            # critical path)
                            kd0 = tk0 - (ntk - TCH // D)
                            if kd0 >= -1:
                                if kd0 >= 0:
                                    nc.vector.tensor_mul(
                                        att2[:], att2[:],
                                        mask_sb[:, kd0 * TCH:(kd0 + 2) * TCH])
                                else:
                                    nc.vector.tensor_mul(
                                        att2[:, TCH:], att2[:, TCH:],
                                        mask_sb[:, 0:TCH])
                            # att column-sum accumulation on DVE (bf16 2x)
                            if pr == 0:
                                nc.vector.tensor_copy(acc[:], att2[:])
                            else:
                                nc.vector.tensor_add(acc[:], acc[:], att2[:])
                            for u in range(2):
                                tk = tk0 + u
                                nc.tensor.matmul(
                                    po[:], v_sb[tk][:, h * D:(h + 1) * D],
                                    att2[:, u * TCH:(u + 1) * TCH],
                                    start=(tk == 0), stop=(tk == ntk - 1))
                        # denominator: one ones-matmul over the accumulated
                        # att sums (contract the 128 acc partitions)
                        pd = ppp.tile([D, TCH], F32, name="pd", tag="pp")
                        nc.tensor.matmul(pd[:], ones_sq[:], acc[:, 0:TCH],
                                         start=True, stop=False)
                        nc.tensor.matmul(pd[:], ones_sq[:], acc[:, TCH:],
                                         start=False, stop=True)
                        recb = normp.tile([D, TCH], F32, name="recb",
                                          tag="recb")
                        nc.vector.reciprocal_approx_fast(recb[:], pd[:])
                        nc.vector.tensor_mul(y_sb[h][:], po[:], recb[:])

                    # ---- P4 for this quarter ----
                    for tq in range(TBQ):
                        pst = post.tile([D, C], BF16, name="pst", tag="pst")
                        for nr in range(NPR):
                            pp = ppp.tile([D, 512], F32, name="pp", tag="pp")
                            for c in range(HPC):
                                nc.tensor.matmul(
                                    pp[:], y_sb[c][:, tq * D:(tq + 1) * D],
                                    wp_all[:, c * C + nr * 512:c * C + (nr + 1) * 512],
                                    start=(c == 0), stop=(c == HPC - 1))
                            nc.vector.tensor_add(
                                pst[:, nr * 512:(nr + 1) * 512], pp[:],
                                bpb[:, nr * 512:(nr + 1) * 512])
                        nc.sync.dma_start(
                            partial_q[ic][tq * D:(tq + 1) * D, :], pst[:])
                        # ---- half-quarter ReduceScatter: issue as soon as
                        # the 2 contributing row-blocks are stored ----
                        if tq % 2 == 1:
                            hf = tq // 2
                            nc.gpsimd.collective_compute(
                                "ReduceScatter",
                                mybir.AluOpType.add,
                                ins=[partial_q[ic][hf * HQ:(hf + 1) * HQ, :]],
                                outs=[rs_q[ic][hf][:, :]],
                                replica_groups=PAIRS,
                            )
                            pending_copies.append((
                                out_ext[ic * RQ + hf * (HQ // 2):
                                        ic * RQ + (hf + 1) * (HQ // 2), :],
                                rs_q[ic][hf][:, :]))
                for (dst, src) in pending_copies:
                    nc.sync.dma_start(dst, src)
    nc.finalize()
    return nc


def _prep_inputs(cfg: Cfg, x, w_attn, b_attn, w_proj, b_proj):
    """Host-side shard/cast/permute. Returns in_maps (list of dicts per core).

    Weight/x layouts are per-block contiguous so that each on-chip block
    loads with a single fully-contiguous DMA.
    """
    T, C, CP, HPC = cfg.T, cfg.C, cfg.CP, cfg.HPC
    NC_CH = C // D
    NJB = 2 * HPC
    VRW = 256
    NVR = CP // VRW
    bf = ml_dtypes.bfloat16
    wq = w_attn[:, 0:C]
    wk = w_attn[:, C:2 * C]
    wvf = w_attn[:, 2 * C:3 * C]
    bq, bk, bvf = b_attn[0:C], b_attn[C:2 * C], b_attn[2 * C:3 * C]

    masks = np.zeros((D, 4 * cfg.TCH), dtype=bf)
    f = np.arange(cfg.TCH)[None, :]
    p = np.arange(D)[:, None]
    for k in range(4):
        keep = (f - p >= 128 * k)
        masks[:, k * cfg.TCH:(k + 1) * cfg.TCH] = np.where(
            keep, 1.0, 0.0).astype(bf)

    in_maps = []
    for core in range(N_CORES):
        b = core // 2
        g = core % 2
        h0 = g * HPC * D            # first col of this head group
        sl = slice(h0, h0 + CP)
        # x[b].T in chunk-contiguous layout: [D, NC_CH*T],
        # xT_p[p, c*T + t] = x[b][t, c*D + p]
        xTc = np.ascontiguousarray(
            x[b].reshape(T, NC_CH, D).transpose(2, 1, 0)).astype(bf)
        xTc = xTc.reshape(D, NC_CH * T)
        wqk_cols = []
        for h in range(HPC):
            hs = slice(h0 + h * D, h0 + (h + 1) * D)
            wqk_cols.append(wq[:, hs])
            wqk_cols.append(wk[:, hs])
        # [C, NJB*D] -> [NJB, D, NC_CH*D]:
        # wqk_c[jb, p, c*D + j] = w[c*D + p, jb*D + j]
        wqk_cat = np.concatenate(wqk_cols, axis=1)
        wqk_c = np.ascontiguousarray(
            wqk_cat.reshape(NC_CH, D, NJB, D).transpose(2, 1, 0, 3)
        ).reshape(NJB, D, NC_CH * D).astype(bf)
        # wv: [C, CP] -> [NVR, D, NC_CH*VRW]
        wv_c = np.ascontiguousarray(
            wvf[:, sl].reshape(NC_CH, D, NVR, VRW).transpose(2, 1, 0, 3)
        ).reshape(NVR, D, NC_CH * VRW).astype(bf)
        # wp: [CP, C] -> [D, HPC*C], wp_p[p, c*C + n] = wp[c*D + p, n]
        wp_c = np.ascontiguousarray(
            w_proj[sl, :].reshape(HPC, D, C).transpose(1, 0, 2)).astype(bf)
        wp_c = wp_c.reshape(D, HPC * C)
        bqk_cols = []
        for h in range(HPC):
            hs = slice(h0 + h * D, h0 + (h + 1) * D)
            bqk_cols.append(bq[hs])
            bqk_cols.append(bk[hs])
        bqk_c = np.ascontiguousarray(np.stack(bqk_cols, axis=1)).astype(np.float32)
        in_maps.append({
            "xT": xTc,
            "wqk": wqk_c,
            "wv": wv_c,
            "wp": wp_c,
            "bqk": bqk_c,
            "bv": np.broadcast_to(
                bvf[sl].reshape(1, CP), (D, CP)).astype(bf),
            "bp": np.broadcast_to(
                (b_proj * (1.0 - g)).reshape(1, C), (D, C)).astype(bf),
            "masks": masks,
        })
    return in_maps


_CFG = Cfg()


def kernel(x, w_attn, b_attn, w_proj, b_proj, _trace=False, _cfg=None):
    from concourse.bass_utils import run_bass_kernel_spmd
    cfg = _cfg or _CFG
    x = np.asarray(x, dtype=np.float32)
    w_attn = np.asarray(w_attn, dtype=np.float32)
    b_attn = np.asarray(b_attn, dtype=np.float32)
    w_proj = np.asarray(w_proj, dtype=np.float32)
    b_proj = np.asarray(b_proj, dtype=np.float32)

    in_maps = _prep_inputs(cfg, x, w_attn, b_attn, w_proj, b_proj)
    nc = build_kernel(cfg)
    res = run_bass_kernel_spmd(nc, in_maps, list(range(N_CORES)), trace=_trace)
    # out rows per core: quarter q, half hf, rank g ->
    #   global rows q*512 + hf*256 + g*128 + [0:128)
    RQ = cfg.TCH // 2
    HQH = RQ // 2  # 128
    outs = []
    for b in range(cfg.B):
        even = res.results[2 * b]["out"].astype(np.float32)
        odd = res.results[2 * b + 1]["out"].astype(np.float32)
        ob = np.empty((cfg.T, cfg.C), dtype=np.float32)
        for q in range(cfg.T // cfg.TCH):
            for hf in range(2):
                r0 = q * RQ + hf * HQH
                g0 = q * cfg.TCH + hf * 2 * HQH
                ob[g0:g0 + HQH] = even[r0:r0 + HQH]
                ob[g0 + HQH:g0 + 2 * HQH] = odd[r0:r0 + HQH]
        outs.append(ob)
    full = np.stack(outs, axis=0)
    if _trace:
        kernel.last_exec_time_ns = res.exec_time_ns
        kernel.last_mean_exec_time_ns = res.mean_exec_time_ns
        kernel.last_scope_times = res.per_core_scope_times
        kernel.last_trace_path = (res.instructions_and_trace[1]
                                  if res.instructions_and_trace else None)
        kernel.last_insts = (res.instructions_and_trace[0]
                             if res.instructions_and_trace else None)
    return full
